# revision 6
# baseline (speedup 1.0000x reference)
"""CropRoi (crop + adaptive max pool 3D) Trainium2 kernel.

Contract: kernel(**inputs) takes the FULL inputs from setup_inputs() and
returns the FULL [N, C, R, R, R] output, distributing work over 8
NeuronCores internally.

Strategy (data-parallel over proposals, f sharded by batch + depth):
  - Proposal crop windows / adaptive-pool bins are data-dependent, so
    the host computes them (bit-exact mirror of the reference float32
    math) and emits a specialized Bass program per core.
  - Batch b of f is assigned to cores (2b, 2b+1); the batch's proposals
    are split between the two cores along d (depth), balanced by
    estimated cost. Each core DMAs one contiguous d-slab of f[b] into
    SBUF (few, huge descriptors), loaded on demand in prefix chunks so
    compute overlaps the load.
  - Per proposal: separable adaptive max-pool, largest axes first,
    straight out of the SBUF slab. Max ops run on DVE (tensor_max over
    strided views, host-coalesced affine runs); pure-copy bins go to
    ScalarE (its own SBUF ports -> true overlap with DVE). Axes with
    L == 7 are identity and skipped.
  - Proposals sharing the same final (smallest) axis spec are grouped:
    the final pooling stage and the output DMA are emitted once per
    group over a shared [C, g, ...] tile.
"""

import sys

sys.path.insert(0, "/opt/trn_rl_repo")

import numpy as np

# Problem constants (hardcoded per spec; kernel.py must be self-contained).
B, C, FS = 4, 64, 32          # f: [B, C, FS, FS, FS] float32
N = 96                        # proposals: [N, 8]
R = 7                         # output pool size
SCALE = 4                     # stride
DIMS_MAX = (32, 32, 32)       # inputs spatial dims (128) // SCALE
N_CORES = 8
MAX_GROUP = 6                 # final-stage group width (SBUF budget)
MAX_SLAB_ROWS = 32            # d-rows cap per core slab


# ----------------------------------------------------------------------------
# Host-side proposal math (bit-exact mirror of reference.py)
# ----------------------------------------------------------------------------

def proposal_params(proposals: np.ndarray):
    """Per proposal: batch index, crop start c0[3], crop end c1[3]."""
    out = []
    f32 = np.float32
    for p in np.asarray(proposals, dtype=np.float32):
        b = int(np.int32(p[0]))
        center, side = p[2:5].astype(f32), p[5:8].astype(f32)
        lo = (center - side / f32(2.0)) / f32(SCALE)
        hi = (center + side / f32(2.0)) / f32(SCALE)
        c0 = np.floor(lo).astype(np.int32)
        c1 = np.ceil(hi).astype(np.int32)
        c0 = np.maximum(c0, 0)
        c1 = np.minimum(c1, np.array(DIMS_MAX, np.int32))
        out.append((b, tuple(int(x) for x in c0), tuple(int(x) for x in c1)))
    return out


def axis_bins(L: int):
    """Adaptive pool bins for length L -> R, relative to crop start."""
    i = np.arange(R)
    starts = (i * L) // R
    ends = ((i + 1) * L + R - 1) // R
    return [(int(s), int(e)) for s, e in zip(starts, ends)]


def coalesce_runs(bins):
    """Group bins into maximal runs of constant (len, start-delta).
    Returns [(i0, cnt, s0, delta, length)]."""
    runs = []
    i = 0
    while i < R:
        s0, e0 = bins[i]
        ln = e0 - s0
        j = i + 1
        delta = None
        while j < R:
            s, e = bins[j]
            if e - s != ln:
                break
            d = s - bins[j - 1][0]
            if delta is None:
                delta = d
            elif d != delta:
                break
            j += 1
        if delta is None:
            delta = 1
        runs.append((i, j - i, s0, delta, ln))
        i = j
    return runs


def pool_seq(L):
    """Pooling order: two largest axes first (desc), smallest last
    (= the grouped final stage)."""
    return sorted(range(3), key=lambda k: (-L[k], k))


def est_cost(c0, c1):
    """Estimated ns of engine time for one proposal (for balancing)."""
    L = [c1[k] - c0[k] for k in range(3)]
    dims = list(L)
    tot = 0.0
    for ax in pool_seq(L):
        if L[ax] == R:
            dims[ax] = R
            continue
        out_elems = 1
        for k in range(3):
            out_elems *= R if k == ax else dims[k]
        for (_, cnt, _, _, ln) in coalesce_runs(axis_bins(L[ax])):
            fd = out_elems // R * cnt
            if ln == 1:
                tot += 190 + 0.45 * fd
            else:
                tot += (ln - 1) * (175 + 1.05 * fd)
        dims[ax] = R
    return tot


# ----------------------------------------------------------------------------
# Bass program builder
# ----------------------------------------------------------------------------

class Sched:
    """Greedy op->engine assignment. Max ops must run on DVE (the only
    fast 2-input engine; GpSimd contends for DVE's 2nd SBUF port so it
    cannot overlap tensor_tensor work). Copies go to ScalarE unless DVE
    is the lighter engine."""

    def __init__(self, nc):
        self.nc = nc
        self.load = {"dve": 0.0, "act": 0.0}

    def tmax(self, out, a, b, fd):
        self.load["dve"] += 175 + 1.05 * fd
        self.nc.vector.tensor_max(out, a, b)

    def copy(self, out, src, fd):
        ca = 195 + 0.45 * fd
        cd = 165 + 0.55 * fd
        if self.load["act"] + ca <= self.load["dve"] + cd:
            self.load["act"] += ca
            self.nc.scalar.copy(out=out, in_=src)
        else:
            self.load["dve"] += cd
            self.nc.vector.tensor_copy(out, src)


def _axslice(t, dim, start, cnt, step):
    """Slice `cnt` elements at `step` along absolute dim of view t.
    step==0 broadcasts."""
    nd = len(t.shape)
    idx = [slice(None)] * nd
    if cnt == 1:
        idx[dim] = slice(start, start + 1)
        return t[tuple(idx)]
    if step == 0:
        idx[dim] = slice(start, start + 1)
        v = t[tuple(idx)]
        shape = list(v.shape)
        shape[dim] = cnt
        return v.broadcast_to(shape)
    idx[dim] = slice(start, start + (cnt - 1) * step + 1, step)
    return t[tuple(idx)]


def _free(v):
    n = 1
    for s in v.shape[1:]:
        n *= s
    return n


def emit_pool(sched, dst, src, dim, bins, src_off=0):
    """Adaptive max-pool along absolute `dim` (dst size R, src size L
    there). Emits run-coalesced tensor_max / copy ops."""
    for (i0, cnt, s0, delta, ln) in coalesce_runs(bins):
        dst_v = _axslice(dst, dim, i0, cnt, 1)
        fd = _free(dst_v)
        base = src_off + s0
        if ln == 1:
            sched.copy(dst_v, _axslice(src, dim, base, cnt, delta), fd)
        else:
            sched.tmax(
                dst_v,
                _axslice(src, dim, base, cnt, delta),
                _axslice(src, dim, base + 1, cnt, delta),
                fd,
            )
            for k in range(2, ln):
                sched.tmax(dst_v, dst_v, _axslice(src, dim, base + k, cnt, delta), fd)


# rearrange patterns collapsing a [C, g, a, b, c] group tile so the group
# axis (gax) becomes absolute dim 2 with everything else merged around it.
_GRP_REARR = {
    0: "p g a b c -> p g a (b c)",
    1: "p g a b c -> p (g a) b c",
    2: "p g a b c -> p (g a b) c",
}


def build_core_program(core):
    """core: dict(n_d, groups). groups: list of lists of props, each
    prop: dict(idx, c0r, c1r) with d coords relative to the slab.
    Input "f": [C, n_d, FS, FS] slab; output "out": [n_slots, C, 343]
    in emission order."""
    import concourse.bacc as bacc
    import concourse.tile as tile
    from concourse import mybir

    n_d = core["n_d"]
    groups = core["groups"]
    n_slots = sum(len(g) for g in groups)

    nc = bacc.Bacc("TRN2", target_bir_lowering=False, debug=False, num_devices=1)
    fs = nc.dram_tensor("f", [C, n_d, FS, FS], mybir.dt.float32, kind="ExternalInput")
    out_dram = nc.dram_tensor(
        "out", [n_slots, C, R * R * R], mybir.dt.float32, kind="ExternalOutput"
    )

    with tile.TileContext(nc) as tc:
        with (
            tc.tile_pool(name="slab", bufs=1) as slab_pool,
            tc.tile_pool(name="x1", bufs=3) as x1_pool,
            tc.tile_pool(name="x2", bufs=2) as x2_pool,
            tc.tile_pool(name="x3", bufs=2) as x3_pool,
        ):
            slab = slab_pool.tile([C, n_d, FS, FS], mybir.dt.float32, tag="slab")
            sched = Sched(nc)
            prefix = 0
            slot = 0

            for grp in groups:
                g = len(grp)
                L0 = [grp[0]["c1r"][k] - grp[0]["c0r"][k] for k in range(3)]
                gax = pool_seq(L0)[2]
                L_last = L0[gax]
                fin_dims = [R, R, R]
                fin_dims[gax] = L_last
                x2g = x2_pool.tile([C, g, *fin_dims], mybir.dt.float32, tag="x2")

                for j, p in enumerate(grp):
                    c0, c1 = p["c0r"], p["c1r"]
                    L = [c1[k] - c0[k] for k in range(3)]
                    need = c1[0]
                    if need > prefix:
                        nc.sync.dma_start(
                            out=slab[:, prefix:need], in_=fs[:, prefix:need]
                        )
                        prefix = need

                    cur = slab[:, c0[0]:c1[0], c0[1]:c1[1], c0[2]:c1[2]]
                    dims = list(L)
                    stages = [ax for ax in pool_seq(L)[:2] if L[ax] != R]
                    for si, ax in enumerate(stages):
                        nd = list(dims)
                        nd[ax] = R
                        if si == len(stages) - 1:
                            dst = x2g[:, j]
                        else:
                            dst = x1_pool.tile([C, *nd], mybir.dt.float32, tag="x1")
                        emit_pool(sched, dst, cur, ax + 1, axis_bins(L[ax]))
                        cur = dst
                        dims = nd
                    if not stages:
                        # both pre-final axes identity: materialize crop
                        sched.copy(x2g[:, j], cur, _free(cur))

                # grouped final stage
                if L_last == R:
                    out_src = x2g
                else:
                    x3g = x3_pool.tile([C, g, R, R, R], mybir.dt.float32, tag="x3")
                    src3 = x2g.rearrange(_GRP_REARR[gax])
                    dst3 = x3g.rearrange(_GRP_REARR[gax])
                    emit_pool(sched, dst3, src3, 2, axis_bins(L_last))
                    out_src = x3g

                nc.sync.dma_start(
                    out=out_dram[slot:slot + g].transpose([1, 0, 2]),
                    in_=out_src.rearrange("p g a b c -> p g (a b c)"),
                )
                slot += g

    nc.compile()
    return nc


# ----------------------------------------------------------------------------
# Host-side planner: batch -> core pair, d-split, grouping
# ----------------------------------------------------------------------------

def plan_cores(params):
    """Returns list of cores: dict(batch, d_lo, n_d, groups, order) where
    order = proposal indices in emission (slot) order."""
    cores = []
    for b in range(B):
        idxs = [i for i, (bb, _, _) in enumerate(params) if bb == b]
        halves = _split_batch(params, idxs)
        for half in halves:
            cores.append(_make_core(params, b, half))
    return cores


def _split_batch(params, idxs):
    if not idxs:
        return [[], []]
    order = sorted(idxs, key=lambda i: params[i][1][0] + params[i][2][0])
    costs = [est_cost(params[i][1], params[i][2]) for i in order]
    best, best_k = None, 1
    for k in range(1, len(order)):
        lo, hi = order[:k], order[k:]
        w_lo = max(params[i][2][0] for i in lo) - min(params[i][1][0] for i in lo)
        w_hi = max(params[i][2][0] for i in hi) - min(params[i][1][0] for i in hi)
        if w_lo > MAX_SLAB_ROWS or w_hi > MAX_SLAB_ROWS:
            continue
        m = max(sum(costs[:k]), sum(costs[k:]))
        if best is None or m < best:
            best, best_k = m, k
    if len(order) == 1:
        return [order, []]
    return [order[:best_k], order[best_k:]]


def _make_core(params, b, idxs):
    if not idxs:
        return {"batch": b, "d_lo": 0, "n_d": 1, "groups": [], "order": []}
    d_lo = min(params[i][1][0] for i in idxs)
    d_hi = max(params[i][2][0] for i in idxs)
    props = []
    for i in idxs:
        _, c0, c1 = params[i]
        props.append({
            "idx": i,
            "c0r": (c0[0] - d_lo, c0[1], c0[2]),
            "c1r": (c1[0] - d_lo, c1[1], c1[2]),
        })
    # group by (final axis, L_last)
    buckets = {}
    for p in props:
        L = [p["c1r"][k] - p["c0r"][k] for k in range(3)]
        gax = pool_seq(L)[2]
        buckets.setdefault((gax, L[gax]), []).append(p)
    groups = []
    for key in sorted(buckets):
        mem = sorted(buckets[key], key=lambda p: p["c1r"][0])
        for s in range(0, len(mem), MAX_GROUP):
            groups.append(mem[s:s + MAX_GROUP])
    # order groups by slab-prefix need (ascending)
    groups.sort(key=lambda grp: max(p["c1r"][0] for p in grp))
    order = [p["idx"] for grp in groups for p in grp]
    return {
        "batch": b, "d_lo": d_lo, "n_d": d_hi - d_lo, "groups": groups,
        "order": order,
    }


# ----------------------------------------------------------------------------
# Top-level kernel
# ----------------------------------------------------------------------------

TRACE = False          # set by test harness to capture NTFF profiles
LAST_RESULTS = None    # list of BassKernelResults when TRACE


def kernel(f, inputs, proposals, cls_ind):
    f = np.ascontiguousarray(np.asarray(f, dtype=np.float32))
    params = proposal_params(proposals)
    cores = plan_cores(params)

    programs = []
    for core in cores:
        if not core["order"]:
            programs.append(None)
            continue
        nc = build_core_program(core)
        slab_np = np.ascontiguousarray(
            f[core["batch"], :, core["d_lo"]:core["d_lo"] + core["n_d"]]
        )
        programs.append((nc, {"f": slab_np}, core["order"]))

    results = _run_programs(programs)

    out = np.empty((N, C, R * R * R), dtype=np.float32)
    for prog, res in zip(programs, results):
        if prog is None:
            continue
        _, _, order = prog
        out[order] = res["out"]
    return out.reshape(N, C, R, R, R)


def _run_programs(programs):
    """Run the per-core programs on the 8 NeuronCores."""
    import jax
    from concourse.bass_utils import run_bass_kernel_spmd

    global LAST_RESULTS
    devices = jax.devices()
    results = []
    raw = []
    for c, prog in enumerate(programs):
        if prog is None:
            results.append(None)
            raw.append(None)
            continue
        nc, in_map, _ = prog
        with jax.default_device(devices[c % len(devices)]):
            res = run_bass_kernel_spmd(nc, [in_map], core_ids=[0], trace=TRACE)
        raw.append(res)
        results.append(res.results[0])
    LAST_RESULTS = raw
    return results


if __name__ == "__main__":
    data = np.load("/tmp/cropref.npz")
    inputs = {
        "f": data["f"], "inputs": data["inputs"],
        "proposals": data["proposals"], "cls_ind": data["cls_ind"],
    }
    exp = data["expected"]
    got = kernel(**inputs)
    err = np.abs(got - exp).max()
    rel = err / max(np.abs(exp).max(), 1e-9)
    print("abs err:", err, "rel err:", rel)


# revision 9
# speedup vs baseline: 1.0116x; 1.0116x over previous
"""CropRoi (crop + adaptive max pool 3D) Trainium2 kernel.

Contract: kernel(**inputs) takes the FULL inputs from setup_inputs() and
returns the FULL [N, C, R, R, R] output, distributing work over 8
NeuronCores internally.

Strategy (data-parallel over proposals, f sharded by batch + depth):
  - Proposal crop windows / adaptive-pool bins are data-dependent, so
    the host computes them (bit-exact mirror of the reference float32
    math) and emits a specialized Bass program per core.
  - All proposals are ordered by (batch, depth) and linearly
    partitioned into 8 cost-balanced chunks; each core DMAs the 1-2
    contiguous d-slabs of f it needs into SBUF (few, huge descriptors,
    issued before all compute so the load overlaps pooling).
  - Per proposal: separable adaptive max-pool, largest axes first,
    straight out of the SBUF slab. Max ops run on DVE (tensor_max /
    max_pool over strided views, host-coalesced affine runs); pure-copy
    bins go to ScalarE (own SBUF ports -> true overlap with DVE). Axes
    with L == 7 are identity and skipped.
  - Proposals sharing the same final (smallest) axis spec are grouped:
    the final pooling stage and the output DMA are emitted once per
    group over a shared [C, g, ...] tile.
"""

import sys

sys.path.insert(0, "/opt/trn_rl_repo")

import numpy as np

# Problem constants (hardcoded per spec; kernel.py must be self-contained).
B, C, FS = 4, 64, 32          # f: [B, C, FS, FS, FS] float32
N = 96                        # proposals: [N, 8]
R = 7                         # output pool size
SCALE = 4                     # stride
DIMS_MAX = (32, 32, 32)       # inputs spatial dims (128) // SCALE
N_CORES = 8
MAX_GROUP = 6                 # final-stage group width (SBUF budget)

USE_POOL_MAX = False           # MAX_POOL instr for len-3 runs (DVE)
GPSIMD_TT_SHARE = 0.0         # fraction of max work to try on GpSimd


# ----------------------------------------------------------------------------
# Host-side proposal math (bit-exact mirror of reference.py)
# ----------------------------------------------------------------------------

def proposal_params(proposals: np.ndarray):
    out = []
    f32 = np.float32
    for p in np.asarray(proposals, dtype=np.float32):
        b = int(np.int32(p[0]))
        center, side = p[2:5].astype(f32), p[5:8].astype(f32)
        lo = (center - side / f32(2.0)) / f32(SCALE)
        hi = (center + side / f32(2.0)) / f32(SCALE)
        c0 = np.floor(lo).astype(np.int32)
        c1 = np.ceil(hi).astype(np.int32)
        c0 = np.maximum(c0, 0)
        c1 = np.minimum(c1, np.array(DIMS_MAX, np.int32))
        out.append((b, tuple(int(x) for x in c0), tuple(int(x) for x in c1)))
    return out


def axis_bins(L: int):
    i = np.arange(R)
    starts = (i * L) // R
    ends = ((i + 1) * L + R - 1) // R
    return [(int(s), int(e)) for s, e in zip(starts, ends)]


def coalesce_runs(bins):
    """[(i0, cnt, s0, delta, length)] maximal affine runs."""
    runs = []
    i = 0
    while i < R:
        s0, e0 = bins[i]
        ln = e0 - s0
        j = i + 1
        delta = None
        while j < R:
            s, e = bins[j]
            if e - s != ln:
                break
            d = s - bins[j - 1][0]
            if delta is None:
                delta = d
            elif d != delta:
                break
            j += 1
        if delta is None:
            delta = 1
        runs.append((i, j - i, s0, delta, ln))
        i = j
    return runs


def pool_seq(L):
    """Pooling order: two largest axes first (desc), smallest last."""
    return sorted(range(3), key=lambda k: (-L[k], k))


def est_cost(c0, c1):
    """Estimated ns of DVE+ACT time for one proposal (balancing)."""
    L = [c1[k] - c0[k] for k in range(3)]
    dims = list(L)
    tot = 0.0
    for ax in pool_seq(L):
        if L[ax] == R:
            dims[ax] = R
            continue
        out_elems = 1
        for k in range(3):
            out_elems *= R if k == ax else dims[k]
        for (_, cnt, _, _, ln) in coalesce_runs(axis_bins(L[ax])):
            fd = out_elems // R * cnt
            if ln == 1:
                tot += 100 + 0.45 * fd
            else:
                tot += (ln - 1) * (170 + 0.9 * fd)
        dims[ax] = R
    return tot


# ----------------------------------------------------------------------------
# Bass program builder
# ----------------------------------------------------------------------------

class Sched:
    """Greedy op->engine assignment with per-engine load tracking."""

    def __init__(self, nc):
        self.nc = nc
        self.load = {"dve": 0.0, "act": 0.0, "gps": 0.0}

    def tmax(self, out, a, b, fd):
        if GPSIMD_TT_SHARE > 0.0:
            cd = 170 + 1.0 * fd
            cg = (170 + 1.0 * fd) / max(GPSIMD_TT_SHARE, 1e-6) * 0.0 + 300 + 1.7 * fd
            if self.load["gps"] + cg < self.load["dve"] + cd:
                self.load["gps"] += cg
                self.nc.gpsimd.tensor_max(out, a, b)
                return
        self.load["dve"] += 170 + 1.0 * fd
        self.nc.vector.tensor_max(out, a, b)

    def pool3(self, out, in5, fd):
        """MAX_POOL reducing innermost dim (len 3) — DVE only."""
        self.load["dve"] += 170 + 3.0 * fd
        self.nc.vector.pool_max(out, in5)

    def copy(self, out, src, fd):
        ca = 200 + 0.45 * fd
        cd = 165 + 0.55 * fd
        if self.load["act"] + ca <= self.load["dve"] + cd:
            self.load["act"] += ca
            self.nc.scalar.copy(out=out, in_=src)
        else:
            self.load["dve"] += cd
            self.nc.vector.tensor_copy(out, src)


def _axslice(t, dim, start, cnt, step):
    nd = len(t.shape)
    idx = [slice(None)] * nd
    if cnt == 1:
        idx[dim] = slice(start, start + 1)
        return t[tuple(idx)]
    if step == 0:
        idx[dim] = slice(start, start + 1)
        v = t[tuple(idx)]
        shape = list(v.shape)
        shape[dim] = cnt
        return v.broadcast_to(shape)
    idx[dim] = slice(start, start + (cnt - 1) * step + 1, step)
    return t[tuple(idx)]


def _free(v):
    n = 1
    for s in v.shape[1:]:
        n *= s
    return n


def _with_inner_dim(v, stride, cnt):
    """Append an innermost [stride, cnt] dim to view v (for MAX_POOL)."""
    import concourse.bass as bass

    ap = [list(p) for p in v.ap] + [[stride, cnt]]
    return bass.AP(tensor=v.tensor, offset=v.offset, ap=ap)


def emit_pool(sched, dst, src, dim, bins):
    """Adaptive max-pool along absolute `dim` (dst R / src L there)."""
    # element stride of src along dim (for pool_max inner dim)
    src_stride = src.ap[dim][0]
    for (i0, cnt, s0, delta, ln) in coalesce_runs(bins):
        dst_v = _axslice(dst, dim, i0, cnt, 1)
        fd = _free(dst_v)
        if ln == 1:
            sched.copy(dst_v, _axslice(src, dim, s0, cnt, delta), fd)
        elif USE_POOL_MAX and ln >= 3 and len(src.shape) <= 4:
            src_v = _axslice(src, dim, s0, cnt, delta)
            sched.pool3(dst_v, _with_inner_dim(src_v, src_stride, ln), fd)
        else:
            sched.tmax(
                dst_v,
                _axslice(src, dim, s0, cnt, delta),
                _axslice(src, dim, s0 + 1, cnt, delta),
                fd,
            )
            for k in range(2, ln):
                sched.tmax(dst_v, dst_v, _axslice(src, dim, s0 + k, cnt, delta), fd)


_GRP_REARR = {
    0: "p g a b c -> p g a (b c)",
    1: "p g a b c -> p (g a) b c",
    2: "p g a b c -> p (g a b) c",
}


def build_core_program(core):
    """core: dict(segments, groups). segments: [(batch, d_lo, n_d)].
    groups: list of lists of props; prop: dict(idx, seg, c0r, c1r) with
    d coords relative to its segment slab. Inputs "f0"["f1"...]:
    [C, n_d, FS, FS] slabs; output "out": [n_slots, C, 343]."""
    import concourse.bacc as bacc
    import concourse.tile as tile
    from concourse import mybir

    segments = core["segments"]
    groups = core["groups"]
    n_slots = sum(len(g) for g in groups)

    nc = bacc.Bacc("TRN2", target_bir_lowering=False, debug=False, num_devices=1)
    fs = [
        nc.dram_tensor(
            f"f{si}", [C, n_d, FS, FS], mybir.dt.float32, kind="ExternalInput"
        )
        for si, (_, _, n_d) in enumerate(segments)
    ]
    out_dram = nc.dram_tensor(
        "out", [n_slots, C, R * R * R], mybir.dt.float32, kind="ExternalOutput"
    )

    with tile.TileContext(nc) as tc:
        with (
            tc.tile_pool(name="slab", bufs=1) as slab_pool,
            tc.tile_pool(name="x1", bufs=4) as x1_pool,
            tc.tile_pool(name="x2", bufs=3) as x2_pool,
            tc.tile_pool(name="x3", bufs=3) as x3_pool,
        ):
            slabs = [
                slab_pool.tile(
                    [C, n_d, FS, FS], mybir.dt.float32,
                    tag=f"slab{si}", name=f"slab{si}",
                )
                for si, (_, _, n_d) in enumerate(segments)
            ]
            sched = Sched(nc)

            # ---- phase 1: all slab chunk DMAs, in first-need order ----
            prefix = [0] * len(segments)
            for grp in groups:
                for p in grp:
                    si, need = p["seg"], p["c1r"][0]
                    if need > prefix[si]:
                        nc.sync.dma_start(
                            out=slabs[si][:, prefix[si]:need],
                            in_=fs[si][:, prefix[si]:need],
                        )
                        prefix[si] = need

            # ---- phase 2: compute + output DMAs ----
            slot = 0
            for grp in groups:
                g = len(grp)
                L0 = [grp[0]["c1r"][k] - grp[0]["c0r"][k] for k in range(3)]
                gax = pool_seq(L0)[2]
                L_last = L0[gax]
                fin_dims = [R, R, R]
                fin_dims[gax] = L_last
                x2g = x2_pool.tile([C, g, *fin_dims], mybir.dt.float32, tag="x2")

                for j, p in enumerate(grp):
                    c0, c1 = p["c0r"], p["c1r"]
                    L = [c1[k] - c0[k] for k in range(3)]
                    cur = slabs[p["seg"]][
                        :, c0[0]:c1[0], c0[1]:c1[1], c0[2]:c1[2]
                    ]
                    dims = list(L)
                    stages = [ax for ax in pool_seq(L)[:2] if L[ax] != R]
                    for si_, ax in enumerate(stages):
                        nd = list(dims)
                        nd[ax] = R
                        if si_ == len(stages) - 1:
                            dst = x2g[:, j]
                        else:
                            dst = x1_pool.tile([C, *nd], mybir.dt.float32, tag="x1")
                        emit_pool(sched, dst, cur, ax + 1, axis_bins(L[ax]))
                        cur = dst
                        dims = nd
                    if not stages:
                        sched.copy(x2g[:, j], cur, _free(cur))

                if L_last == R:
                    out_src = x2g
                else:
                    x3g = x3_pool.tile([C, g, R, R, R], mybir.dt.float32, tag="x3")
                    src3 = x2g.rearrange(_GRP_REARR[gax])
                    dst3 = x3g.rearrange(_GRP_REARR[gax])
                    emit_pool(sched, dst3, src3, 2, axis_bins(L_last))
                    out_src = x3g

                nc.sync.dma_start(
                    out=out_dram[slot:slot + g].transpose([1, 0, 2]),
                    in_=out_src.rearrange("p g a b c -> p g (a b c)"),
                )
                slot += g

    nc.compile()
    return nc


# ----------------------------------------------------------------------------
# Host-side planner: global linear partition over (batch, d)-sorted stream
# ----------------------------------------------------------------------------

def plan_cores(params):
    """Returns per-core dicts (segments, groups, order)."""
    stream = []  # (batch, d_center, idx) sorted
    for b in range(B):
        idxs = [i for i, (bb, _, _) in enumerate(params) if bb == b]
        idxs.sort(key=lambda i: params[i][1][0] + params[i][2][0])
        stream.extend(idxs)
    costs = [est_cost(params[i][1], params[i][2]) for i in stream]
    cuts = _balanced_cuts(costs, N_CORES)

    cores = []
    for c in range(N_CORES):
        chunk = stream[cuts[c]:cuts[c + 1]]
        cores.append(_make_core(params, chunk))
    return cores


def _balanced_cuts(costs, k):
    """Linear partition of costs into k contiguous chunks minimizing max
    chunk sum (greedy threshold + refinement)."""
    n = len(costs)
    total = sum(costs)
    prefix = np.concatenate([[0.0], np.cumsum(costs)])

    def feasible(cap):
        cuts = [0]
        cur = 0
        for _ in range(k):
            # furthest j with sum(cur..j) <= cap
            j = int(np.searchsorted(prefix, prefix[cur] + cap, side="right")) - 1
            if j <= cur:
                j = cur + 1  # at least one item
            cuts.append(min(j, n))
            cur = min(j, n)
        return cuts if cur >= n else None

    lo, hi = max(costs), total
    for _ in range(40):
        mid = (lo + hi) / 2
        if feasible(mid):
            hi = mid
        else:
            lo = mid
    cuts = feasible(hi)
    while len(cuts) < k + 1:
        cuts.append(n)
    return cuts


def _make_core(params, idxs):
    if not idxs:
        return {"segments": [], "groups": [], "order": []}
    # segments: contiguous d-windows per batch present in this chunk
    seg_map = {}
    segments = []
    for i in idxs:
        b = params[i][0]
        if b not in seg_map:
            seg_map[b] = len(segments)
            segments.append(b)
    seg_info = []
    for b in segments:
        mine = [i for i in idxs if params[i][0] == b]
        d_lo = min(params[i][1][0] for i in mine)
        d_hi = max(params[i][2][0] for i in mine)
        seg_info.append((b, d_lo, d_hi - d_lo))

    props = []
    for i in idxs:
        b, c0, c1 = params[i]
        si = seg_map[b]
        d_lo = seg_info[si][1]
        props.append({
            "idx": i, "seg": si,
            "c0r": (c0[0] - d_lo, c0[1], c0[2]),
            "c1r": (c1[0] - d_lo, c1[1], c1[2]),
        })

    # group by (final axis, L_last)
    buckets = {}
    for p in props:
        L = [p["c1r"][k] - p["c0r"][k] for k in range(3)]
        gax = pool_seq(L)[2]
        buckets.setdefault((gax, L[gax]), []).append(p)
    groups = []
    for key in sorted(buckets):
        mem = sorted(buckets[key], key=lambda p: (p["seg"], p["c1r"][0]))
        for s in range(0, len(mem), MAX_GROUP):
            groups.append(mem[s:s + MAX_GROUP])
    groups.sort(key=lambda grp: max(p["seg"] * 100 + p["c1r"][0] for p in grp))
    order = [p["idx"] for grp in groups for p in grp]
    return {"segments": seg_info, "groups": groups, "order": order}


# ----------------------------------------------------------------------------
# Top-level kernel
# ----------------------------------------------------------------------------

TRACE = False
LAST_RESULTS = None


def kernel(f, inputs, proposals, cls_ind):
    f = np.ascontiguousarray(np.asarray(f, dtype=np.float32))
    params = proposal_params(proposals)
    cores = plan_cores(params)

    programs = []
    for core in cores:
        if not core["order"]:
            programs.append(None)
            continue
        nc = build_core_program(core)
        in_map = {}
        for si, (b, d_lo, n_d) in enumerate(core["segments"]):
            in_map[f"f{si}"] = np.ascontiguousarray(f[b, :, d_lo:d_lo + n_d])
        programs.append((nc, in_map, core["order"]))

    results = _run_programs(programs)

    out = np.empty((N, C, R * R * R), dtype=np.float32)
    for prog, res in zip(programs, results):
        if prog is None:
            continue
        _, _, order = prog
        out[order] = res["out"]
    return out.reshape(N, C, R, R, R)


def _run_programs(programs):
    import jax
    from concourse.bass_utils import run_bass_kernel_spmd

    global LAST_RESULTS
    devices = jax.devices()
    results = []
    raw = []
    for c, prog in enumerate(programs):
        if prog is None:
            results.append(None)
            raw.append(None)
            continue
        nc, in_map, _ = prog
        with jax.default_device(devices[c % len(devices)]):
            res = run_bass_kernel_spmd(nc, [in_map], core_ids=[0], trace=TRACE)
        raw.append(res)
        results.append(res.results[0])
    LAST_RESULTS = raw
    return results


if __name__ == "__main__":
    data = np.load("/tmp/cropref.npz")
    inputs = {
        "f": data["f"], "inputs": data["inputs"],
        "proposals": data["proposals"], "cls_ind": data["cls_ind"],
    }
    exp = data["expected"]
    got = kernel(**inputs)
    err = np.abs(got - exp).max()
    rel = err / max(np.abs(exp).max(), 1e-9)
    print("abs err:", err, "rel err:", rel)


# revision 10
# speedup vs baseline: 1.1923x; 1.1786x over previous
"""CropRoi (crop + adaptive max pool 3D) Trainium2 kernel.

Contract: kernel(**inputs) takes the FULL inputs from setup_inputs() and
returns the FULL [N, C, R, R, R] output, distributing work over 8
NeuronCores internally.

Strategy (data-parallel over proposals, f sharded by batch + depth):
  - Proposal crop windows / adaptive-pool bins are data-dependent, so
    the host computes them (bit-exact mirror of the reference float32
    math) and emits a specialized Bass program per core.
  - All proposals are ordered by (batch, depth) and linearly
    partitioned into 8 cost-balanced chunks; each core DMAs the 1-2
    contiguous d-slabs of f it needs into SBUF (few, huge descriptors,
    issued before all compute so the load overlaps pooling).
  - Per proposal: separable adaptive max-pool, largest axes first,
    straight out of the SBUF slab. Max ops run on DVE (tensor_max /
    max_pool over strided views, host-coalesced affine runs); pure-copy
    bins go to ScalarE (own SBUF ports -> true overlap with DVE). Axes
    with L == 7 are identity and skipped.
  - Proposals sharing the same final (smallest) axis spec are grouped:
    the final pooling stage and the output DMA are emitted once per
    group over a shared [C, g, ...] tile.
"""

import sys

sys.path.insert(0, "/opt/trn_rl_repo")

import numpy as np

# Problem constants (hardcoded per spec; kernel.py must be self-contained).
B, C, FS = 4, 64, 32          # f: [B, C, FS, FS, FS] float32
N = 96                        # proposals: [N, 8]
R = 7                         # output pool size
SCALE = 4                     # stride
DIMS_MAX = (32, 32, 32)       # inputs spatial dims (128) // SCALE
N_CORES = 8
MAX_GROUP = 6                 # final-stage group width (SBUF budget)

USE_POOL_MAX = False           # MAX_POOL instr for len-3 runs (DVE)
GPSIMD_TT_SHARE = 0.0         # fraction of max work to try on GpSimd


# ----------------------------------------------------------------------------
# Host-side proposal math (bit-exact mirror of reference.py)
# ----------------------------------------------------------------------------

def proposal_params(proposals: np.ndarray):
    out = []
    f32 = np.float32
    for p in np.asarray(proposals, dtype=np.float32):
        b = int(np.int32(p[0]))
        center, side = p[2:5].astype(f32), p[5:8].astype(f32)
        lo = (center - side / f32(2.0)) / f32(SCALE)
        hi = (center + side / f32(2.0)) / f32(SCALE)
        c0 = np.floor(lo).astype(np.int32)
        c1 = np.ceil(hi).astype(np.int32)
        c0 = np.maximum(c0, 0)
        c1 = np.minimum(c1, np.array(DIMS_MAX, np.int32))
        out.append((b, tuple(int(x) for x in c0), tuple(int(x) for x in c1)))
    return out


def axis_bins(L: int):
    i = np.arange(R)
    starts = (i * L) // R
    ends = ((i + 1) * L + R - 1) // R
    return [(int(s), int(e)) for s, e in zip(starts, ends)]


def coalesce_runs(bins):
    """[(i0, cnt, s0, delta, length)] maximal affine runs."""
    runs = []
    i = 0
    while i < R:
        s0, e0 = bins[i]
        ln = e0 - s0
        j = i + 1
        delta = None
        while j < R:
            s, e = bins[j]
            if e - s != ln:
                break
            d = s - bins[j - 1][0]
            if delta is None:
                delta = d
            elif d != delta:
                break
            j += 1
        if delta is None:
            delta = 1
        runs.append((i, j - i, s0, delta, ln))
        i = j
    return runs


def pool_seq(L):
    """Pooling order: two largest axes first (desc), smallest last."""
    return sorted(range(3), key=lambda k: (-L[k], k))


def est_cost(c0, c1):
    """Estimated ns of DVE+ACT time for one proposal (balancing)."""
    L = [c1[k] - c0[k] for k in range(3)]
    dims = list(L)
    tot = 0.0
    for ax in pool_seq(L):
        if L[ax] == R:
            dims[ax] = R
            continue
        out_elems = 1
        for k in range(3):
            out_elems *= R if k == ax else dims[k]
        for (_, cnt, _, _, ln) in coalesce_runs(axis_bins(L[ax])):
            fd = out_elems // R * cnt
            if ln == 1:
                tot += 0.25 * (100 + 0.45 * fd)
            else:
                tot += (ln - 1) * (170 + 0.9 * fd)
        dims[ax] = R
    return tot


# ----------------------------------------------------------------------------
# Bass program builder
# ----------------------------------------------------------------------------

class Sched:
    """Greedy op->engine assignment with per-engine load tracking."""

    def __init__(self, nc):
        self.nc = nc
        self.load = {"dve": 0.0, "act": 0.0, "gps": 0.0}

    def tmax(self, out, a, b, fd):
        if GPSIMD_TT_SHARE > 0.0:
            cd = 170 + 1.0 * fd
            cg = (170 + 1.0 * fd) / max(GPSIMD_TT_SHARE, 1e-6) * 0.0 + 300 + 1.7 * fd
            if self.load["gps"] + cg < self.load["dve"] + cd:
                self.load["gps"] += cg
                self.nc.gpsimd.tensor_max(out, a, b)
                return
        self.load["dve"] += 170 + 1.0 * fd
        self.nc.vector.tensor_max(out, a, b)

    def pool3(self, out, in5, fd):
        """MAX_POOL reducing innermost dim (len 3) — DVE only."""
        self.load["dve"] += 170 + 3.0 * fd
        self.nc.vector.pool_max(out, in5)

    def copy(self, out, src, fd):
        ca = 200 + 0.45 * fd
        cd = 165 + 0.55 * fd
        if self.load["act"] + ca <= self.load["dve"] + cd:
            self.load["act"] += ca
            self.nc.scalar.copy(out=out, in_=src)
        else:
            self.load["dve"] += cd
            self.nc.vector.tensor_copy(out, src)


def _axslice(t, dim, start, cnt, step):
    nd = len(t.shape)
    idx = [slice(None)] * nd
    if cnt == 1:
        idx[dim] = slice(start, start + 1)
        return t[tuple(idx)]
    if step == 0:
        idx[dim] = slice(start, start + 1)
        v = t[tuple(idx)]
        shape = list(v.shape)
        shape[dim] = cnt
        return v.broadcast_to(shape)
    idx[dim] = slice(start, start + (cnt - 1) * step + 1, step)
    return t[tuple(idx)]


def _free(v):
    n = 1
    for s in v.shape[1:]:
        n *= s
    return n


def _with_inner_dim(v, stride, cnt):
    """Append an innermost [stride, cnt] dim to view v (for MAX_POOL)."""
    import concourse.bass as bass

    ap = [list(p) for p in v.ap] + [[stride, cnt]]
    return bass.AP(tensor=v.tensor, offset=v.offset, ap=ap)


def emit_pool(sched, dst, src, dim, bins):
    """Adaptive max-pool along absolute `dim` (dst R / src L there)."""
    # element stride of src along dim (for pool_max inner dim)
    src_stride = src.ap[dim][0]
    for (i0, cnt, s0, delta, ln) in coalesce_runs(bins):
        dst_v = _axslice(dst, dim, i0, cnt, 1)
        fd = _free(dst_v)
        if ln == 1:
            sched.copy(dst_v, _axslice(src, dim, s0, cnt, delta), fd)
        elif USE_POOL_MAX and ln >= 3 and len(src.shape) <= 4:
            src_v = _axslice(src, dim, s0, cnt, delta)
            sched.pool3(dst_v, _with_inner_dim(src_v, src_stride, ln), fd)
        else:
            sched.tmax(
                dst_v,
                _axslice(src, dim, s0, cnt, delta),
                _axslice(src, dim, s0 + 1, cnt, delta),
                fd,
            )
            for k in range(2, ln):
                sched.tmax(dst_v, dst_v, _axslice(src, dim, s0 + k, cnt, delta), fd)


_GRP_REARR = {
    0: "p g a b c -> p g a (b c)",
    1: "p g a b c -> p (g a) b c",
    2: "p g a b c -> p (g a b) c",
}


def build_core_program(core):
    """core: dict(segments, groups). segments: [(batch, d_lo, n_d)].
    groups: list of lists of props; prop: dict(idx, seg, c0r, c1r) with
    d coords relative to its segment slab. Inputs "f0"["f1"...]:
    [C, n_d, FS, FS] slabs; output "out": [n_slots, C, 343]."""
    import concourse.bacc as bacc
    import concourse.tile as tile
    from concourse import mybir

    segments = core["segments"]
    groups = core["groups"]
    n_slots = sum(len(g) for g in groups)

    nc = bacc.Bacc("TRN2", target_bir_lowering=False, debug=False, num_devices=1)
    fs = [
        nc.dram_tensor(
            f"f{si}", [C, n_d, n_h, FS], mybir.dt.float32, kind="ExternalInput"
        )
        for si, (_, _, n_d, _, n_h) in enumerate(segments)
    ]
    out_dram = nc.dram_tensor(
        "out", [n_slots, C, R * R * R], mybir.dt.float32, kind="ExternalOutput"
    )

    with tile.TileContext(nc) as tc:
        with (
            tc.tile_pool(name="slab", bufs=1) as slab_pool,
            tc.tile_pool(name="x1", bufs=6) as x1_pool,
            tc.tile_pool(name="x2", bufs=3) as x2_pool,
            tc.tile_pool(name="x3", bufs=3) as x3_pool,
        ):
            slabs = [
                slab_pool.tile(
                    [C, n_d, n_h, FS], mybir.dt.float32,
                    tag=f"slab{si}", name=f"slab{si}",
                )
                for si, (_, _, n_d, _, n_h) in enumerate(segments)
            ]
            sched = Sched(nc)

            # ---- phase 1: all slab chunk DMAs, in first-need order ----
            prefix = [0] * len(segments)
            for grp in groups:
                for p in grp:
                    si, need = p["seg"], p["c1r"][0]
                    if need > prefix[si]:
                        nc.sync.dma_start(
                            out=slabs[si][:, prefix[si]:need],
                            in_=fs[si][:, prefix[si]:need],
                        )
                        prefix[si] = need

            # ---- phase 2: compute + output DMAs ----
            slot = 0
            for grp in groups:
                g = len(grp)
                L0 = [grp[0]["c1r"][k] - grp[0]["c0r"][k] for k in range(3)]
                gax = pool_seq(L0)[2]
                L_last = L0[gax]
                fin_dims = [R, R, R]
                fin_dims[gax] = L_last
                x2g = x2_pool.tile([C, g, *fin_dims], mybir.dt.float32, tag="x2")

                for j, p in enumerate(grp):
                    c0, c1 = p["c0r"], p["c1r"]
                    L = [c1[k] - c0[k] for k in range(3)]
                    cur = slabs[p["seg"]][
                        :, c0[0]:c1[0], c0[1]:c1[1], c0[2]:c1[2]
                    ]
                    dims = list(L)
                    stages = [ax for ax in pool_seq(L)[:2] if L[ax] != R]
                    for si_, ax in enumerate(stages):
                        nd = list(dims)
                        nd[ax] = R
                        if si_ == len(stages) - 1:
                            dst = x2g[:, j]
                        else:
                            dst = x1_pool.tile([C, *nd], mybir.dt.float32, tag="x1")
                        emit_pool(sched, dst, cur, ax + 1, axis_bins(L[ax]))
                        cur = dst
                        dims = nd
                    if not stages:
                        sched.copy(x2g[:, j], cur, _free(cur))

                if L_last == R:
                    out_src = x2g
                else:
                    x3g = x3_pool.tile([C, g, R, R, R], mybir.dt.float32, tag="x3")
                    src3 = x2g.rearrange(_GRP_REARR[gax])
                    dst3 = x3g.rearrange(_GRP_REARR[gax])
                    emit_pool(sched, dst3, src3, 2, axis_bins(L_last))
                    out_src = x3g

                nc.sync.dma_start(
                    out=out_dram[slot:slot + g].transpose([1, 0, 2]),
                    in_=out_src.rearrange("p g a b c -> p g (a b c)"),
                )
                slot += g

    nc.compile()
    return nc


# ----------------------------------------------------------------------------
# Host-side planner: global linear partition over (batch, d)-sorted stream
# ----------------------------------------------------------------------------

def plan_cores(params):
    """Returns per-core dicts (segments, groups, order)."""
    stream = []  # (batch, d_center, idx) sorted
    for b in range(B):
        idxs = [i for i, (bb, _, _) in enumerate(params) if bb == b]
        idxs.sort(key=lambda i: params[i][1][0] + params[i][2][0])
        stream.extend(idxs)
    costs = [est_cost(params[i][1], params[i][2]) for i in stream]
    cuts = _balanced_cuts(costs, N_CORES)

    cores = []
    for c in range(N_CORES):
        chunk = stream[cuts[c]:cuts[c + 1]]
        cores.append(_make_core(params, chunk))
    return cores


def _balanced_cuts(costs, k):
    """Linear partition of costs into k contiguous chunks minimizing max
    chunk sum (greedy threshold + refinement)."""
    n = len(costs)
    total = sum(costs)
    prefix = np.concatenate([[0.0], np.cumsum(costs)])

    def feasible(cap):
        cuts = [0]
        cur = 0
        for _ in range(k):
            # furthest j with sum(cur..j) <= cap
            j = int(np.searchsorted(prefix, prefix[cur] + cap, side="right")) - 1
            if j <= cur:
                j = cur + 1  # at least one item
            cuts.append(min(j, n))
            cur = min(j, n)
        return cuts if cur >= n else None

    lo, hi = max(costs), total
    for _ in range(40):
        mid = (lo + hi) / 2
        if feasible(mid):
            hi = mid
        else:
            lo = mid
    cuts = feasible(hi)
    while len(cuts) < k + 1:
        cuts.append(n)
    return cuts


def _make_core(params, idxs):
    if not idxs:
        return {"segments": [], "groups": [], "order": []}
    # segments: contiguous d-windows per batch present in this chunk
    seg_map = {}
    segments = []
    for i in idxs:
        b = params[i][0]
        if b not in seg_map:
            seg_map[b] = len(segments)
            segments.append(b)
    seg_info = []
    for b in segments:
        mine = [i for i in idxs if params[i][0] == b]
        d_lo = min(params[i][1][0] for i in mine)
        d_hi = max(params[i][2][0] for i in mine)
        h_lo = min(params[i][1][1] for i in mine)
        h_hi = max(params[i][2][1] for i in mine)
        seg_info.append((b, d_lo, d_hi - d_lo, h_lo, h_hi - h_lo))

    props = []
    for i in idxs:
        b, c0, c1 = params[i]
        si = seg_map[b]
        d_lo, h_lo = seg_info[si][1], seg_info[si][3]
        props.append({
            "idx": i, "seg": si,
            "c0r": (c0[0] - d_lo, c0[1] - h_lo, c0[2]),
            "c1r": (c1[0] - d_lo, c1[1] - h_lo, c1[2]),
        })

    # group by (final axis, L_last)
    buckets = {}
    for p in props:
        L = [p["c1r"][k] - p["c0r"][k] for k in range(3)]
        gax = pool_seq(L)[2]
        buckets.setdefault((gax, L[gax]), []).append(p)
    groups = []
    for key in sorted(buckets):
        mem = sorted(buckets[key], key=lambda p: (p["seg"], p["c1r"][0]))
        for s in range(0, len(mem), MAX_GROUP):
            groups.append(mem[s:s + MAX_GROUP])
    groups.sort(key=lambda grp: max(p["seg"] * 100 + p["c1r"][0] for p in grp))
    order = [p["idx"] for grp in groups for p in grp]
    return {"segments": seg_info, "groups": groups, "order": order}


# ----------------------------------------------------------------------------
# Top-level kernel
# ----------------------------------------------------------------------------

TRACE = False
LAST_RESULTS = None


def kernel(f, inputs, proposals, cls_ind):
    f = np.ascontiguousarray(np.asarray(f, dtype=np.float32))
    params = proposal_params(proposals)
    cores = plan_cores(params)

    programs = []
    for core in cores:
        if not core["order"]:
            programs.append(None)
            continue
        nc = build_core_program(core)
        in_map = {}
        for si, (b, d_lo, n_d, h_lo, n_h) in enumerate(core["segments"]):
            in_map[f"f{si}"] = np.ascontiguousarray(
                f[b, :, d_lo:d_lo + n_d, h_lo:h_lo + n_h]
            )
        programs.append((nc, in_map, core["order"]))

    results = _run_programs(programs)

    out = np.empty((N, C, R * R * R), dtype=np.float32)
    for prog, res in zip(programs, results):
        if prog is None:
            continue
        _, _, order = prog
        out[order] = res["out"]
    return out.reshape(N, C, R, R, R)


def _run_programs(programs):
    import jax
    from concourse.bass_utils import run_bass_kernel_spmd

    global LAST_RESULTS
    devices = jax.devices()
    results = []
    raw = []
    for c, prog in enumerate(programs):
        if prog is None:
            results.append(None)
            raw.append(None)
            continue
        nc, in_map, _ = prog
        with jax.default_device(devices[c % len(devices)]):
            res = run_bass_kernel_spmd(nc, [in_map], core_ids=[0], trace=TRACE)
        raw.append(res)
        results.append(res.results[0])
    LAST_RESULTS = raw
    return results


if __name__ == "__main__":
    data = np.load("/tmp/cropref.npz")
    inputs = {
        "f": data["f"], "inputs": data["inputs"],
        "proposals": data["proposals"], "cls_ind": data["cls_ind"],
    }
    exp = data["expected"]
    got = kernel(**inputs)
    err = np.abs(got - exp).max()
    rel = err / max(np.abs(exp).max(), 1e-9)
    print("abs err:", err, "rel err:", rel)
